# revision 49
# baseline (speedup 1.0000x reference)
"""Cross-attention kernel for Trainium2, 8 NeuronCores.

Sharding: data parallel over batch (B=2) x tensor parallel over heads
(16 heads -> 4 per core). Core c handles batch c//4, heads 4*(c%4) ..
4*(c%4)+3. Each core:
  1. projects q -> qp^T, kv -> kp^T / vp for its head shard
     (transposed layout: contraction dim on partitions),
  2. computes scores^T = kp^T.T @ qp^T per head (kv on partitions,
     q on free axis), exp via ACT (softmax scale folded into the
     activation's scale), max-free softmax,
  3. PV with an all-ones column appended to vp so the softmax
     denominator falls out of the same matmul (row 64 of the PV psum),
  4. normalizes via reciprocal + PE broadcast + DVE multiply,
  5. applies its slice of the output projection, producing a partial
     [Nq, C] result that the host sums across the 4 cores of a batch.

All matmuls run in float32r (full PE rate for N>=256, ~1e-3 max
elementwise rounding vs fp32). f32r matmuls are self-loading (no
LDWEIGHTS) and can carry only ONE semaphore wait, so every fresh PSUM
accumulation group starts with a tiny bf16 dummy matmul that absorbs
the WAR wait on the psum slot, and weight tiles are "pre-consumed" by
dummy matmuls right after their DMA so later matmuls never wait on
them.

Persistent intermediates (qpT/kpT/aoT) are split into per-region tiles
so attention unblocks as soon as the exact region it reads is drained.
PSUM: "big" pool (2x [128,1024]) for QK score pairs and the output
projection; "proj" pool (2x [128,512]) shared by q/k projection
accumulators, v-projection accumulators, and the softmax-denominator
broadcast; "pv" pool (2x [65,512]) for PV accumulators.
"""

import sys

if "/opt/trn_rl_repo" not in sys.path:
    sys.path.insert(0, "/opt/trn_rl_repo")

import numpy as np

import concourse.bass as bass
import concourse.tile as tile
from concourse import bacc, mybir
from concourse.bass_utils import run_bass_kernel_spmd

F32 = mybir.dt.float32
F32R = mybir.dt.float32r
BF16 = mybir.dt.bfloat16
EXP = mybir.ActivationFunctionType.Exp

C = 1024
H = 16
D = 64
SCALE = D ** (-0.5)
H_LOC = 4          # heads per core
CL = H_LOC * D     # 256 local projection width
N_CORES = 8
KT = C // 128      # 8 contraction tiles


def _dummy(nc, ps_ap, src_ap):
    """Tiny bf16 matmul writing ps_ap[0:2,0:2], reading 4 bytes of
    src_ap. Absorbs one semaphore wait so the following f32r matmul
    stays at <=1 wait."""
    w = src_ap[0:1, 0:1].bitcast(BF16)
    nc.tensor.matmul(ps_ap[0:2, 0:2], w, w, start=True, stop=True)


def build(nq=2048, nkv=2048):
    nc = bacc.Bacc("TRN2", target_bir_lowering=False, debug=False)

    qT = nc.dram_tensor("qT", [C, nq], F32, kind="ExternalInput").ap().bitcast(F32R)
    kvT = nc.dram_tensor("kvT", [C, nkv], F32, kind="ExternalInput").ap().bitcast(F32R)
    wq_d = nc.dram_tensor("wq", [C, CL], F32, kind="ExternalInput").ap().bitcast(F32R)
    wkv_d = nc.dram_tensor("wkv", [C, 2 * CL], F32, kind="ExternalInput").ap().bitcast(F32R)
    wout_d = nc.dram_tensor("wout", [CL, C], F32, kind="ExternalInput").ap().bitcast(F32R)
    bq_d = nc.dram_tensor("bq", [CL, 1], F32, kind="ExternalInput").ap()
    bk_d = nc.dram_tensor("bk", [CL, 1], F32, kind="ExternalInput").ap()
    bv_d = nc.dram_tensor("bv", [1, CL], F32, kind="ExternalInput").ap().bitcast(F32R)
    out_d = nc.dram_tensor("out", [nq, C], F32, kind="ExternalOutput").ap()

    nkt = nkv // 128   # kv row tiles (16)
    nqb = nq // 512    # query blocks (4)
    nkb = nkv // 512   # kv column blocks for kp (4)

    with tile.TileContext(nc) as tc:
        with (
            nc.allow_low_precision(reason="f32r matmul inputs"),
            tc.tile_pool(name="weights", bufs=1) as wpool,
            tc.tile_pool(name="persist", bufs=1) as ppool,
            tc.tile_pool(name="stream", bufs=6) as spool,
            tc.tile_pool(name="kvstream", bufs=12) as kvpool,
            tc.tile_pool(name="exps", bufs=10) as epool,
            tc.tile_pool(name="small", bufs=3) as smpool,
            tc.tile_pool(name="outs", bufs=4) as opool,
            tc.tile_pool(name="big_ps", bufs=2, space="PSUM") as big_ps,
            tc.tile_pool(name="proj_ps", bufs=2, space="PSUM") as proj_ps,
            tc.tile_pool(name="pv_ps", bufs=2, space="PSUM") as pv_ps,
        ):
            # ---- constants & weights -------------------------------
            ones_f = wpool.tile([128, 4], F32)
            nc.vector.memset(ones_f[:], 1.0)
            ones_r = wpool.tile([128, 4], F32R, tag="ones_r")
            nc.vector.tensor_copy(ones_r[:], ones_f[:])
            onesrow_f = wpool.tile([1, 128], F32, tag="onesrow_f")
            nc.vector.memset(onesrow_f[:], 1.0)
            onesrow_r = wpool.tile([1, 128], F32R, tag="onesrow_r")
            nc.vector.tensor_copy(onesrow_r[:], onesrow_f[:])

            wkv_all = wpool.tile([128, KT * 2 * CL], F32R, tag="wkv_all")
            HK = KT // 2
            for hh in range(2):
                nc.sync.dma_start(
                    wkv_all[:, hh * HK * 2 * CL:(hh + 1) * HK * 2 * CL]
                    .rearrange("p (k c) -> p k c", k=HK),
                    wkv_d[hh * HK * 128:(hh + 1) * HK * 128, :]
                    .rearrange("(k p) c -> p k c", p=128),
                )
            wkv_sb = [wkv_all[:, k * 2 * CL:(k + 1) * 2 * CL] for k in range(KT)]
            bq_sb = []
            bk_sb = []
            for m in range(2):
                t = wpool.tile([128, 1], F32, tag=f"bk_sb{m}", name=f"bk_sb{m}")
                nc.sync.dma_start(t[:], bk_d[m * 128:(m + 1) * 128, :])
                bk_sb.append(t)
            bv_sb = wpool.tile([1, CL], F32R, tag="bv_sb")
            nc.sync.dma_start(bv_sb[:], bv_d)

            # pre-consume attention-ramp-critical weights on PE; wout is
            # DMA'd and pre-consumed later (first needed by out-proj)
            scratch = pv_ps.tile([65, 512], F32, tag="pv")
            for t in wkv_sb + [bv_sb]:
                _dummy(nc, scratch[:], t[:])
            wout_sb = []

            wq_sb = []

            def load_wq():
                wq_all = wpool.tile([128, KT * CL], F32R, tag="wq_all")
                nc.sync.dma_start(
                    wq_all[:].rearrange("p (k c) -> p k c", k=KT),
                    wq_d.rearrange("(k p) c -> p k c", p=128),
                )
                wq_sb.extend(wq_all[:, k * CL:(k + 1) * CL] for k in range(KT))
                _dummy(nc, scratch[:], wq_all[:])
                for m in range(2):
                    t = wpool.tile([128, 1], F32, tag=f"bq_sb{m}",
                                   name=f"bq_sb{m}")
                    nc.sync.dma_start(t[:], bq_d[m * 128:(m + 1) * 128, :])
                    bq_sb.append(t)

            def load_wout():
                for k2 in range(2):
                    t = wpool.tile([128, C], F32R, tag=f"wout_sb{k2}",
                                   name=f"wout_sb{k2}")
                    nc.sync.dma_start(t[:], wout_d[k2 * 128:(k2 + 1) * 128, :])
                    wout_sb.append(t)
                for t in wout_sb:
                    _dummy(nc, scratch[:], t[:])

            # warm the ACT exp table early (one-time ~2.7us load)
            warm = wpool.tile([1, 2], F32, tag="warm")
            nc.scalar.activation(warm[:], ones_f[0:1, 0:2], EXP)

            # ---- persistent projection outputs (per-region tiles) --
            kpT = [[ppool.tile([128, 512], F32R, tag=f"kpT{m}_{nb}",
                               name=f"kpT{m}_{nb}") for nb in range(nkb)]
                   for m in range(2)]
            qpT = [[ppool.tile([128, 512], F32R, tag=f"qpT{m}_{qb}",
                               name=f"qpT{m}_{qb}") for qb in range(nqb)]
                   for m in range(2)]
            vp_aug = [ppool.tile([128, 260], F32R, tag=f"vp{t}", name=f"vp{t}")
                      for t in range(nkt)]
            aoT = [[ppool.tile([128, 512], F32R, tag=f"aoT{p}_{qb}",
                               name=f"aoT{p}_{qb}") for qb in range(nqb)]
                   for p in range(2)]

            def q_pass(qb, dma_eng=None):
                """qpT[m][qb] = (wq[:, m].T @ qT[:, qb]) + bq[m]; one big
                psum tile holds both m-halves."""
                dma_eng = dma_eng or nc.sync
                ps = big_ps.tile([128, 1024], F32, tag="big", name="qps")
                _dummy(nc, ps[:], ones_r[:])
                for k in range(KT):
                    ch = spool.tile([128, 512], F32R, tag="qch", name="qch")
                    dma_eng.dma_start(
                        ch[:], qT[k * 128:(k + 1) * 128,
                                  qb * 512:(qb + 1) * 512]
                    )
                    for m in range(2):
                        nc.tensor.matmul(
                            ps[:, m * 512:(m + 1) * 512],
                            wq_sb[k][:, m * 128:(m + 1) * 128],
                            ch[:],
                            start=(k == 0),
                            stop=(k == KT - 1),
                        )
                for m in range(2):
                    nc.vector.tensor_scalar_add(
                        qpT[m][qb][:], ps[:, m * 512:(m + 1) * 512], bq_sb[m][:]
                    )

            def kp_vp_split(nb):
                """kp + vp for kv cols [nb*512,+512) as two injectable
                closures sharing one chunk read: part A streams the chunks
                and does kp into the two proj slots; part B re-reads the
                still-resident chunks for vp (two kvt-pair accumulators,
                sequential on the same slots). Keeps the big pool QK-only."""
                chunks = []

                def part_a():
                    ps = []
                    for m in range(2):
                        t = proj_ps.tile([128, 512], F32, tag="proj",
                                         name=f"kponly{m}")
                        _dummy(nc, t[:], ones_r[:])
                        ps.append(t)
                    for k in range(KT):
                        ch = kvpool.tile([128, 512], F32R, tag="kvch", name="kvch")
                        chunks.append(ch)
                        nc.sync.dma_start(
                            ch[:], kvT[k * 128:(k + 1) * 128,
                                       nb * 512:(nb + 1) * 512]
                        )
                        for m in range(2):
                            nc.tensor.matmul(
                                ps[m][:],
                                wkv_sb[k][:, m * 128:(m + 1) * 128],
                                ch[:],
                                start=(k == 0),
                                stop=(k == KT - 1),
                            )
                    for m in range(2):
                        nc.vector.tensor_scalar_add(
                            kpT[m][nb][:], ps[m][:], bk_sb[m][:]
                        )

                def part_b():
                    for j in range(2):
                        t = proj_ps.tile([128, 512], F32, tag="proj",
                                         name=f"vps{j}")
                        for r in range(2):
                            nc.tensor.matmul(
                                t[:, r * 256:(r + 1) * 256],
                                onesrow_r[:], bv_sb[:],
                                start=True, stop=False,
                            )
                        for k in range(KT):
                            for r in range(2):
                                nc.tensor.matmul(
                                    t[:, r * 256:(r + 1) * 256],
                                    chunks[k][:, (2 * j + r) * 128:
                                              (2 * j + r + 1) * 128],
                                    wkv_sb[k][:, CL:2 * CL],
                                    start=False,
                                    stop=(k == KT - 1),
                                )
                        for r in range(2):
                            kvt = 4 * nb + 2 * j + r
                            dst = vp_aug[kvt]
                            nc.vector.tensor_copy(
                                dst[:].rearrange("p (h c) -> p h c",
                                                 h=H_LOC)[:, :, 0:64],
                                t[:, r * 256:(r + 1) * 256]
                                .rearrange("p (h c) -> p h c", c=64),
                            )
                            nc.vector.tensor_copy(
                                dst[:].rearrange("p (h c) -> p h c",
                                                 h=H_LOC)[:, :, 64:65],
                                ones_r[:, 0:4].rearrange("p (h c) -> p h c",
                                                         c=1),
                            )

                return part_a, part_b

            def vp_block(kvt):
                """v-projection for one kv row-tile via a column-block
                re-read of kvT; one proj-pool slot."""
                blk = spool.tile([128, 1024], F32R, tag="vpbl", name="vpbl")
                vsrc = kvT[:, kvt * 128:(kvt + 1) * 128].rearrange(
                    "(k p) c -> p k c", p=128
                )
                nc.sync.dma_start(
                    blk[:].rearrange("p (k c) -> p k c", k=KT), vsrc
                )
                ps = proj_ps.tile([128, CL], F32, tag="proj", name="vpps")
                nc.tensor.matmul(
                    ps[:], onesrow_r[:], bv_sb[:], start=True, stop=False
                )
                for k in range(KT):
                    nc.tensor.matmul(
                        ps[:],
                        blk[:, k * 128:(k + 1) * 128],
                        wkv_sb[k][:, CL:2 * CL],
                        start=False,
                        stop=(k == KT - 1),
                    )
                dst = vp_aug[kvt]
                nc.vector.tensor_copy(
                    dst[:].rearrange("p (h c) -> p h c", h=H_LOC)[:, :, 0:64],
                    ps[:].rearrange("p (h c) -> p h c", c=64),
                )
                nc.vector.tensor_copy(
                    dst[:].rearrange("p (h c) -> p h c", h=H_LOC)[:, :, 64:65],
                    ones_r[:, 0:4].rearrange("p (h c) -> p h c", c=1),
                )

            def kv_pass(nb):
                """Single read of kvT cols [nb*512,+512): kp accumulates in
                one big tile (m0|m1); vp for the 4 kv row-tiles accumulates
                in two proj tiles (2 kv-tiles of 256 cols each)."""
                kps = big_ps.tile([128, 1024], F32, tag="big", name="kps")
                _dummy(nc, kps[:], ones_r[:])
                vps = []
                for j in range(2):
                    t = proj_ps.tile([128, 512], F32, tag="proj", name=f"vps{j}")
                    # bias row via K=1 ones matmuls; first also absorbs the
                    # WAR wait on the slot
                    for r in range(2):
                        nc.tensor.matmul(
                            t[:, r * 256:(r + 1) * 256],
                            onesrow_r[:], bv_sb[:], start=True, stop=False,
                        )
                    vps.append(t)
                for k in range(KT):
                    ch = kvpool.tile([128, 512], F32R, tag="kvch", name="kvch")
                    nc.sync.dma_start(
                        ch[:], kvT[k * 128:(k + 1) * 128,
                                   nb * 512:(nb + 1) * 512]
                    )
                    for m in range(2):
                        nc.tensor.matmul(
                            kps[:, m * 512:(m + 1) * 512],
                            wkv_sb[k][:, m * 128:(m + 1) * 128],
                            ch[:],
                            start=(k == 0),
                            stop=(k == KT - 1),
                        )
                    for j in range(2):
                        for r in range(2):
                            nc.tensor.matmul(
                                vps[j][:, r * 256:(r + 1) * 256],
                                ch[:, (2 * j + r) * 128:(2 * j + r + 1) * 128],
                                wkv_sb[k][:, CL:2 * CL],
                                start=False,
                                stop=(k == KT - 1),
                            )
                for m in range(2):
                    nc.vector.tensor_scalar_add(
                        kpT[m][nb][:], kps[:, m * 512:(m + 1) * 512], bk_sb[m][:]
                    )
                for j in range(2):
                    for r in range(2):
                        kvt = 4 * nb + 2 * j + r
                        dst = vp_aug[kvt]
                        nc.vector.tensor_copy(
                            dst[:].rearrange("p (h c) -> p h c", h=H_LOC)[:, :, 0:64],
                            vps[j][:, r * 256:(r + 1) * 256]
                            .rearrange("p (h c) -> p h c", c=64),
                        )
                        nc.vector.tensor_copy(
                            dst[:].rearrange("p (h c) -> p h c", h=H_LOC)[:, :, 64:65],
                            ones_r[:, 0:4].rearrange("p (h c) -> p h c", c=1),
                        )

            def attention_qb(qb, inject=None, deferred=None):
                """inject: {kvt: fn} emitted inside pair 0's loop (kv/q
                passes self-paced against the attention stream). deferred:
                out-projection closures of the previous query block,
                spread across pair 0's loop."""
                inject = inject or {}
                deferred = list(deferred or [])
                for pair in range(2):
                    pv = [pv_ps.tile([65, 512], F32, tag="pv", name=f"pvps{h}")
                          for h in range(2)]
                    exs = {}
                    # first QK+exp goes ahead of the PV-slot dummies so the
                    # in-order PE stream is not head-of-line blocked on the
                    # previous pair's drain chain
                    for kvt in range(nkt):
                        qk = big_ps.tile([128, 1024], F32, tag="big", name="qkps")
                        for h in range(2):
                            nc.tensor.matmul(
                                qk[:, h * 512:(h + 1) * 512],
                                kpT[pair][kvt // 4][h * 64:(h + 1) * 64,
                                                    (kvt % 4) * 128:
                                                    (kvt % 4) * 128 + 128],
                                qpT[pair][qb][h * 64:(h + 1) * 64, :],
                                start=True,
                                stop=True,
                            )
                        ex = epool.tile([128, 1024], F32R, tag="exp", name="exp")
                        nc.scalar.activation(ex[:], qk[:], EXP, scale=SCALE)
                        if kvt == 0:
                            for h in range(2):
                                _dummy(nc, pv[h][:], ones_r[:])
                        for h in range(2):
                            nc.tensor.matmul(
                                pv[h][:],
                                vp_aug[kvt][:, (2 * pair + h) * 65:
                                            (2 * pair + h) * 65 + 65],
                                ex[:, h * 512:(h + 1) * 512],
                                start=(kvt == 0),
                                stop=(kvt == nkt - 1),
                            )
                        if pair == 0:
                            if kvt in inject:
                                inject[kvt]()
                            if kvt % 4 == 3 and deferred:
                                deferred.pop(0)()
                    for h in range(2):
                        r = smpool.tile([1, 512], F32R, tag="recip", name="recip")
                        nc.vector.reciprocal(r[:], pv[h][64:65, :])
                        pv_sb = smpool.tile([64, 512], F32, tag="pv_sb", name="pv_sb")
                        nc.vector.tensor_copy(pv_sb[:], pv[h][0:64, :])
                        bc = proj_ps.tile([64, 512], F32, tag="proj", name="bcps")
                        nc.tensor.matmul(
                            bc[:], onesrow_r[0:1, 0:64], r[:],
                            start=True, stop=True,
                        )
                        nc.vector.tensor_mul(
                            aoT[pair][qb][h * 64:(h + 1) * 64, :],
                            bc[:],
                            pv_sb[:],
                        )
                # out-projection closures: emitted inside the NEXT query
                # block's attention (or flushed at the very end)
                def make_op(mt, qb=qb):
                    def emit(use_act=False):
                        o_sb = opool.tile([128, 1024], F32, tag="o_sb", name="o_sb")
                        for nb in range(2):
                            ops = proj_ps.tile([128, 512], F32, tag="proj",
                                               name="outps")
                            _dummy(nc, ops[:], ones_r[:])
                            for k2 in range(2):
                                nc.tensor.matmul(
                                    ops[:],
                                    aoT[k2][qb][:, mt * 128:(mt + 1) * 128],
                                    wout_sb[k2][:, nb * 512:(nb + 1) * 512],
                                    start=(k2 == 0),
                                    stop=(k2 == 1),
                                )
                            if use_act:
                                nc.scalar.copy(
                                    o_sb[:, nb * 512:(nb + 1) * 512], ops[:]
                                )
                            else:
                                nc.vector.tensor_copy(
                                    o_sb[:, nb * 512:(nb + 1) * 512], ops[:]
                                )
                        nc.sync.dma_start(
                            out_d[qb * 512 + mt * 128:
                                  qb * 512 + (mt + 1) * 128, :],
                            o_sb[:],
                        )
                    return emit
                return [make_op(mt) for mt in range(4)]

            # ---- emission order ------------------------------------
            # First kv/q block up front, then the remaining projection
            # passes and previous-block out-projections are injected into
            # the attention stream, self-paced against big-pool slots.
            kv_pass(0)
            load_wq()
            q_pass(0)
            a1, b1 = kp_vp_split(1)
            a2, b2 = kp_vp_split(2)
            a3, b3 = kp_vp_split(3)
            op = attention_qb(0, inject={
                0: a1,
                1: b1,
                3: load_wout,
                4: a2,
                5: b2,
                8: a3,
                9: b3,
                13: lambda: q_pass(1),
            })
            op = attention_qb(1, inject={
                1: lambda: q_pass(2),
                9: lambda: q_pass(3),
            }, deferred=op)
            op = attention_qb(2, deferred=op)
            op = attention_qb(3, deferred=op)
            for g in op:
                g(use_act=True)

    nc.compile()
    return nc


_CACHE = {}


def _get_nc(nq, nkv):
    key = (nq, nkv)
    if key not in _CACHE:
        _CACHE[key] = build(nq, nkv)
    return _CACHE[key]


def _prep_in_maps(q, kv, w_q, b_q, w_kv, b_kv, w_out, b_out):
    B = q.shape[0]
    in_maps = []
    qT = [np.ascontiguousarray(q[b].T) for b in range(B)]
    kvT = [np.ascontiguousarray(kv[b].T) for b in range(B)]
    for c in range(N_CORES):
        b, g = divmod(c, 4)
        s = slice(CL * g, CL * (g + 1))
        in_maps.append({
            "qT": qT[b],
            "kvT": kvT[b],
            "wq": np.ascontiguousarray(w_q[:, s]),
            "wkv": np.ascontiguousarray(
                np.concatenate([w_kv[:, s], w_kv[:, C + CL * g: C + CL * (g + 1)]], axis=1)
            ),
            "wout": np.ascontiguousarray(w_out[s, :]),
            "bq": np.ascontiguousarray(b_q[s].reshape(CL, 1)),
            "bk": np.ascontiguousarray(b_kv[s].reshape(CL, 1)),
            "bv": np.ascontiguousarray(
                b_kv[C + CL * g: C + CL * (g + 1)].reshape(1, CL)
            ),
        })
    return in_maps


def run(q, kv, w_q, b_q, w_kv, b_kv, w_out, b_out, trace=False):
    q = np.asarray(q, dtype=np.float32)
    kv = np.asarray(kv, dtype=np.float32)
    B, nq, _ = q.shape
    nkv = kv.shape[1]
    nc = _get_nc(nq, nkv)
    in_maps = _prep_in_maps(
        q, kv,
        np.asarray(w_q, np.float32), np.asarray(b_q, np.float32),
        np.asarray(w_kv, np.float32), np.asarray(b_kv, np.float32),
        np.asarray(w_out, np.float32), np.asarray(b_out, np.float32),
    )
    res = run_bass_kernel_spmd(nc, in_maps, list(range(N_CORES)), trace=trace)
    out = np.zeros((B, nq, C), dtype=np.float32)
    for c in range(N_CORES):
        b = c // 4
        out[b] += res.results[c]["out"]
    out += np.asarray(b_out, np.float32)[None, None, :]
    return out, res


def kernel(**inputs):
    out, _ = run(**inputs)
    return out
